# revision 14
# baseline (speedup 1.0000x reference)
# RBF Gram matrix kernel for Trainium2 (8 NeuronCores, SPMD).
#
# reference:  G[i, j] = exp(-gamma * ||x_i - y_j||^2)
#
# Factorization used on device:
#   G[i, j] = exp(2*gamma*xy[i,j] - gamma*||x_i||^2) * exp(-gamma*||y_j||^2)
#             \------------- ACT (bias per row i) --/   \-- DVE row mult --/
#
# which maps perfectly onto the engines:
#   PE  : xy = x_c @ y^T in fp8(e4m3) with perf_mode=DoubleRow
#         (K_virt=256 per MM -> ~1.8x bf16 MM throughput)
#   ACT : o = Exp(scale*psum + bias_i), PSUM->SBUF bf16, 2048-wide chunks
#         (bias = -gamma*||x_i||^2 is per-partition, so no DVE add needed)
#   DVE : o2 = o * c_j  (c_j = exp(-gamma*||y_j||^2) row, bf16 2x mode)
#   DMA : o2 (fp8) -> DRAM; host upcasts to fp32
#
# Sharding: row-shard x across 8 cores (1024 rows each), replicate y.
#
# Note: the exp/exp split assumes the intermediate exp(2g*xy - g*x2) does
# not overflow, which holds for the standardized inputs this kernel serves
# (|2g*xy| << g*x2). Inputs are quantized to fp8 at scale 16; the 1/256
# factor is folded into the ACT scale immediate.
import os

import numpy as np
import ml_dtypes

N_CORES = 8
N_FULL = 8192          # rows of x (and of G)
M_FULL = 8192          # rows of y (cols of G)
D = 512                # feature dim (contraction)
MC = N_FULL // N_CORES # 1024 rows of x per core
P = 128                # SBUF partitions
KT = D // P            # 4 k-subtiles of 128
MT = MC // P           # 8 m-tiles per core
CW = 2048              # chunk width (ACT/DVE/psum-slot granularity)
JC = M_FULL // CW      # 4 j-chunks
NN = CW // 512         # 4 matmul slices of 512 per chunk
G = JC * MT            # 32 chunks per core
XS = 16.0              # fp8 input scale (folded out via ACT scale)

_cache = {}


def _build_program(scale_imm: float, out_fp8: bool, dve_mode: str):
    """Raw-Bass build: explicit per-engine programs + hand-rolled semaphores."""
    from contextlib import ExitStack, contextmanager

    import concourse.bass as bass
    import concourse.mybir as mybir
    from concourse import bacc

    class _NoBarrierBlock(bass.BassBlock):
        """BassBlock whose exit emits per-engine drains but no all-engine
        barrier; cross-engine ordering is fully covered by our semaphores."""

        def __exit__(self, exc_type, exc_val, exc_tb):
            if exc_type is not None:
                return
            for engine, last_body in self.last_body.items():
                with self.bass.body(last_body, parent=self.bass.cur_bb,
                                    allow_existing_parent=True):
                    engine.br(self.end_bb)
            self.bass.switch_bb(self.end_bb)
            gpsimd_type = self.bass.gpsimd.engine
            for eng_type, eng in self.bass.engines.items():
                if eng_type == gpsimd_type:
                    continue
                dr = mybir.InstDrain(
                    name=self.bass.get_next_instruction_name(),
                    ins=[], outs=[], bass_is_fusable=False)
                dr.engine = eng_type
                eng.add_instruction(dr)

    @contextmanager
    def _no_barrier_block(nc):
        assert nc.cur_block is None
        blk = _NoBarrierBlock(nc, f"block_{nc.next_id()}")
        nc.cur_block = blk
        try:
            with blk:
                yield blk
        finally:
            nc.cur_block = None

    NWARM = 32
    O_SLOTS = 4            # ACT output staging slots (bf16)
    O2_SLOTS = 6           # DVE output staging slots (fp8/bf16)
    fp8 = mybir.dt.float8e4
    odt = mybir.dt.float8e4 if out_fp8 else mybir.dt.bfloat16
    two_step = dve_mode == "mul16copy8" and out_fp8

    nc = bacc.Bacc("TRN2", target_bir_lowering=False, debug=False,
                   num_devices=N_CORES)

    x_d = nc.dram_tensor("x8", [P, MT * KT, P], fp8,
                         kind="ExternalInput").ap()
    y_d = nc.dram_tensor("y8", [P, JC * KT, CW], fp8,
                         kind="ExternalInput").ap()
    c_d = nc.dram_tensor("cb", [P, M_FULL], mybir.dt.bfloat16,
                         kind="ExternalInput").ap()
    x2_d = nc.dram_tensor("x2b", [P, MT], mybir.dt.float32,
                          kind="ExternalInput").ap()
    out_d = nc.dram_tensor("out", [MC, M_FULL], odt,
                           kind="ExternalOutput").ap()

    with ExitStack() as ctx:
        ec = ctx.enter_context
        x_sb = ec(nc.sbuf_tensor([P, MT * KT, P], fp8))
        y_sb = ec(nc.sbuf_tensor([P, JC * KT, CW], fp8))
        c_sb = ec(nc.sbuf_tensor([P, M_FULL], mybir.dt.bfloat16))
        x2_sb = ec(nc.sbuf_tensor([P, MT], mybir.dt.float32))
        scr_sb = ec(nc.sbuf_tensor([P, 2 * P], mybir.dt.bfloat16))
        o_sb = ec(nc.sbuf_tensor([P, O_SLOTS, CW], mybir.dt.bfloat16))
        o2_sb = ec(nc.sbuf_tensor([P, O2_SLOTS, CW], odt))
        o3_sb = (ec(nc.sbuf_tensor("o3_sb", [P, CW], mybir.dt.bfloat16))
                 if two_step else None)
        ps = ec(nc.psum_tensor([P, 2, CW], mybir.dt.float32))

        s_scr = ec(nc.semaphore(name="s_scr"))
        s_x = ec(nc.semaphore(name="s_x"))
        s_x2 = ec(nc.semaphore(name="s_x2"))
        s_y = [ec(nc.semaphore(name=f"s_y{i}")) for i in range(JC)]
        s_y0k = [ec(nc.semaphore(name=f"s_y0k{i}")) for i in range(KT)]
        s_c = [ec(nc.semaphore(name=f"s_c{i}")) for i in range(JC)]
        s_mm = ec(nc.semaphore(name="s_mm"))
        s_act = ec(nc.semaphore(name="s_act"))
        s_dve = ec(nc.semaphore(name="s_dve"))
        s_osl = [ec(nc.semaphore(name=f"s_osl{i}")) for i in range(O2_SLOTS)]

        def lhsT(m, ko):
            return x_sb[:, m * KT + 2 * ko:m * KT + 2 * ko + 2, :]

        def rhs(jc, ko, nn):
            return y_sb[:, jc * KT + 2 * ko:jc * KT + 2 * ko + 2,
                        nn * 512:(nn + 1) * 512]

        # plan: chunks 0 and G-1 are split into NN 512-wide pieces so the
        # pipeline head fills (and the tail drains) at piece granularity.
        # cum[(g, i)] = cumulative semaphore count after piece i of chunk g
        # (the same piece structure is used for s_mm, s_act and s_dve).
        pieces = [NN if g in (0, G - 1) else 1 for g in range(G)]
        cum, tot = {}, {}
        _c = 0
        for g in range(G):
            for i in range(pieces[g]):
                _c += 1
                cum[(g, i)] = _c
            tot[g] = _c
        # s_osl[sl] count accumulated by out-DMAs of chunks < g on slot sl
        osl_before = {}
        _oc = [0] * O2_SLOTS
        for g in range(G):
            osl_before[g] = _oc[g % O2_SLOTS]
            _oc[g % O2_SLOTS] += 16 * pieces[g]

        def psl(g, i):
            w = CW // pieces[g]
            return slice(i * w, (i + 1) * w)

        with _no_barrier_block(nc) as block:

            @block.sync
            def _(sync):
                # startup set, in critical-path order; y chunk 0 is split
                # per k-subtile so its pieces ride parallel DMA queues.
                sync.dma_start(out=x_sb[:], in_=x_d).then_inc(s_x, 16)
                for kt in range(KT):
                    sync.dma_start(out=y_sb[:, kt:kt + 1, :],
                                   in_=y_d[:, kt:kt + 1, :]
                                   ).then_inc(s_y0k[kt], 16)
                sync.dma_start(out=x2_sb[:], in_=x2_d).then_inc(s_x2, 16)
                sync.dma_start(out=c_sb[:, 0:CW],
                               in_=c_d[:, 0:CW]).then_inc(s_c[0], 16)
                sync.dma_start(out=y_sb[:, KT:2 * KT, :],
                               in_=y_d[:, KT:2 * KT, :]).then_inc(s_y[1], 16)
                sync.dma_start(out=c_sb[:, CW:2 * CW],
                               in_=c_d[:, CW:2 * CW]).then_inc(s_c[1], 16)
                for g in range(G):
                    jc, m = g // MT, g % MT
                    # just-in-time prefetch of later y/c chunks
                    if g == 2:
                        sync.dma_start(out=y_sb[:, 2 * KT:3 * KT, :],
                                       in_=y_d[:, 2 * KT:3 * KT, :]
                                       ).then_inc(s_y[2], 16)
                    if g == 4:
                        sync.dma_start(out=c_sb[:, 2 * CW:3 * CW],
                                       in_=c_d[:, 2 * CW:3 * CW]
                                       ).then_inc(s_c[2], 16)
                    if g == 10:
                        sync.dma_start(out=y_sb[:, 3 * KT:4 * KT, :],
                                       in_=y_d[:, 3 * KT:4 * KT, :]
                                       ).then_inc(s_y[3], 16)
                    if g == 12:
                        sync.dma_start(out=c_sb[:, 3 * CW:4 * CW],
                                       in_=c_d[:, 3 * CW:4 * CW]
                                       ).then_inc(s_c[3], 16)
                    sl = g % O2_SLOTS
                    msl = slice(m * P, (m + 1) * P)
                    for i in range(pieces[g]):
                        sync.wait_ge(s_dve, cum[(g, i)])
                        w = psl(g, i)
                        sync.dma_start(
                            out=out_d[msl, jc * CW + w.start:
                                      jc * CW + w.stop],
                            in_=o2_sb[:, sl, w]).then_inc(s_osl[sl], 16)
                # the end-of-block DRAIN quiesces the DGE queues

            @block.tensor
            def _(tensor):
                # PE warm-up: keep the HAM activity window busy while the
                # startup DMAs land, so real matmuls run at 2.4 GHz.
                tensor.wait_ge(s_scr, 1)
                for _ in range(NWARM):
                    tensor.matmul(ps[:, 0, 0:P], lhsT=scr_sb[:, P:2 * P],
                                  rhs=scr_sb[:, 0:P], start=True, stop=True)
                tensor.wait_ge(s_x, 16)
                for g in range(G):
                    jc, m = g // MT, g % MT
                    sl = g % 2
                    if g >= 2:
                        tensor.wait_ge(s_act, tot[g - 2])   # psum slot free
                    if m == 0 and jc > 0:
                        tensor.wait_ge(s_y[jc], 16)
                    if g == 0:
                        # nn-outer so the first ACT piece unblocks after 2
                        # MMs; per-kt gating because the 4 startup y DMAs
                        # ride parallel queues and can land out of order
                        for nn in range(NN):
                            for ko in range(2):
                                if nn == 0:
                                    tensor.wait_ge(s_y0k[2 * ko], 16)
                                    tensor.wait_ge(s_y0k[2 * ko + 1], 16)
                                inst = tensor.matmul(
                                    ps[:, sl, nn * 512:(nn + 1) * 512],
                                    lhsT=lhsT(m, ko),
                                    rhs=rhs(jc, ko, nn),
                                    start=(ko == 0),
                                    stop=(ko == 1),
                                    perf_mode=mybir.MatmulPerfMode.DoubleRow,
                                )
                            inst.then_inc(s_mm, 1)
                    else:
                        for ko in range(2):
                            for nn in range(NN):
                                inst = tensor.matmul(
                                    ps[:, sl, nn * 512:(nn + 1) * 512],
                                    lhsT=lhsT(m, ko),
                                    rhs=rhs(jc, ko, nn),
                                    start=(ko == 0),
                                    stop=(ko == 1),
                                    perf_mode=mybir.MatmulPerfMode.DoubleRow,
                                )
                        if pieces[g] == 1:
                            inst.then_inc(s_mm, 1)
                        else:
                            inst.then_inc(s_mm, pieces[g])

            @block.scalar
            def _(scalar):
                # dummy activation so the one-time exp table load (~2.7us)
                # overlaps the startup DMAs instead of the first real chunk.
                # o_sb slot 0 is private to this engine until chunk 0.
                scalar.activation(o_sb[:, 0, 0:2], o_sb[:, 0, 0:2],
                                  mybir.ActivationFunctionType.Exp)
                scalar.wait_ge(s_x2, 16)
                for g in range(G):
                    jc, m = g // MT, g % MT
                    osl = g % O_SLOTS
                    if g >= O_SLOTS:
                        scalar.wait_ge(s_dve, tot[g - O_SLOTS])  # o slot free
                    for i in range(pieces[g]):
                        scalar.wait_ge(s_mm, cum[(g, i)])
                        w = psl(g, i)
                        scalar.activation(
                            o_sb[:, osl, w], ps[:, g % 2, w],
                            mybir.ActivationFunctionType.Exp,
                            bias=x2_sb[:, m:m + 1],
                            scale=float(scale_imm)).then_inc(s_act, 1)

            @block.vector
            def _(vector):
                vector.memset(scr_sb[:], 0.0).then_inc(s_scr, 1)
                for g in range(G):
                    jc, m = g // MT, g % MT
                    osl = g % O_SLOTS
                    sl = g % O2_SLOTS
                    if g >= O2_SLOTS:
                        vector.wait_ge(s_osl[sl], osl_before[g])  # slot free
                    if m == 0:
                        vector.wait_ge(s_c[jc], 16)
                    csl = c_sb[:, jc * CW:(jc + 1) * CW]
                    for i in range(pieces[g]):
                        vector.wait_ge(s_act, cum[(g, i)])
                        w = psl(g, i)
                        if two_step:
                            # keep the multiply in the bf16 2x mode; the
                            # fp8 downcast rides the copy uop
                            vector.tensor_mul(o3_sb[:, w], o_sb[:, osl, w],
                                              csl[:, w])
                            vector.tensor_copy(
                                o2_sb[:, sl, w],
                                o3_sb[:, w]).then_inc(s_dve, 1)
                        else:
                            vector.tensor_mul(
                                o2_sb[:, sl, w], o_sb[:, osl, w],
                                csl[:, w]).then_inc(s_dve, 1)

        nc.compile()
    return nc


def _pack_xT(x_8: np.ndarray) -> np.ndarray:
    """[MC, D] fp8 -> SBUF image [128, MT*KT, 128]; k-subtile kt of m-tile m
    at dim1 index m*KT+kt with element [p, ., c] = x[m*128 + c, kt*128 + p]."""
    mcc, d = x_8.shape
    mt, kt = mcc // P, d // P
    a = x_8.reshape(mt, P, kt, P)          # [m, c, kt, p]
    a = a.transpose(3, 0, 2, 1)            # [p, m, kt, c]
    return np.ascontiguousarray(a.reshape(P, mt * kt, P))


def _pack_yT(y_8: np.ndarray) -> np.ndarray:
    """[M, D] fp8 -> SBUF image [128, JC*KT, CW]; k-subtile kt of j-chunk jc
    at dim1 index jc*KT+kt with element [p, ., c] = y[jc*CW + c, kt*128 + p]."""
    m, d = y_8.shape
    jc, kt = m // CW, d // P
    a = y_8.reshape(jc, CW, kt, P)         # [jc, c, kt, p]
    a = a.transpose(3, 0, 2, 1)            # [p, jc, kt, c]
    return np.ascontiguousarray(a.reshape(P, jc * kt, CW))


def kernel(x: np.ndarray, y: np.ndarray, gamma: np.ndarray) -> np.ndarray:
    from concourse.bass_utils import run_bass_kernel_spmd

    x = np.asarray(x, dtype=np.float32)
    y = np.asarray(y, dtype=np.float32)
    g = float(np.asarray(gamma))

    n, d = x.shape
    m = y.shape[0]
    assert (n, d, m) == (N_FULL, D, M_FULL), (n, d, m)

    out_fp8 = os.environ.get("RBF_OUT", "fp8") == "fp8"
    dve_mode = os.environ.get("RBF_DVE", "mul16copy8")
    scale_imm = 2.0 * g / (XS * XS)
    key = (g, out_fp8, dve_mode)
    if key not in _cache:
        _cache.clear()
        _cache[key] = _build_program(scale_imm, out_fp8, dve_mode)
    nc = _cache[key]

    # host-side prep (O(N*D), ~0.01% of kernel FLOPs)
    f8 = ml_dtypes.float8_e4m3fn
    x8 = np.clip(x * XS, -240.0, 240.0).astype(f8)
    y8 = np.clip(y * XS, -240.0, 240.0).astype(f8)
    y_img = _pack_yT(y8)
    x2 = np.einsum("nd,nd->n", x, x, dtype=np.float64)
    y2 = np.einsum("md,md->m", y, y, dtype=np.float64)
    c_row = np.exp(-g * y2).astype(ml_dtypes.bfloat16)
    c_rep = np.ascontiguousarray(np.broadcast_to(c_row[None, :], (P, m)))

    in_maps = []
    for c in range(N_CORES):
        sl = slice(c * MC, (c + 1) * MC)
        x2_c = np.ascontiguousarray(
            (-g * x2[sl]).astype(np.float32).reshape(MT, P).T)   # [128, MT]
        in_maps.append({"x8": _pack_xT(x8[sl]), "y8": y_img,
                        "cb": c_rep, "x2b": x2_c})

    trace = bool(int(os.environ.get("RBF_TRACE", "0")))
    res = run_bass_kernel_spmd(nc, in_maps, core_ids=list(range(N_CORES)),
                               trace=trace)
    global LAST_RESULTS
    LAST_RESULTS = res
    return np.concatenate(
        [r["out"].astype(np.float32) for r in res.results], axis=0)


LAST_RESULTS = None


# revision 18
# speedup vs baseline: 1.0034x; 1.0034x over previous
# RBF Gram matrix kernel for Trainium2 (8 NeuronCores, SPMD).
#
# reference:  G[i, j] = exp(-gamma * ||x_i - y_j||^2)
#
# Factorization used on device:
#   G[i, j] = exp(2*gamma*xy[i,j] - gamma*||x_i||^2) * exp(-gamma*||y_j||^2)
#             \------------- ACT (bias per row i) --/   \-- DVE row mult --/
#
# which maps perfectly onto the engines:
#   PE  : xy = x_c @ y^T in fp8(e4m3) with perf_mode=DoubleRow
#         (K_virt=256 per MM -> ~1.8x bf16 MM throughput)
#   ACT : o = Exp(scale*psum + bias_i), PSUM->SBUF bf16, 2048-wide chunks
#         (bias = -gamma*||x_i||^2 is per-partition, so no DVE add needed)
#   DVE : o2 = o * c_j  (c_j = exp(-gamma*||y_j||^2) row, bf16 2x mode)
#   DMA : o2 (fp8) -> DRAM; host upcasts to fp32
#
# Sharding: row-shard x across 8 cores (1024 rows each), replicate y.
#
# Note: the exp/exp split assumes the intermediate exp(2g*xy - g*x2) does
# not overflow, which holds for the standardized inputs this kernel serves
# (|2g*xy| << g*x2). Inputs are quantized to fp8 at scale 16; the 1/256
# factor is folded into the ACT scale immediate.
import os

import numpy as np
import ml_dtypes

N_CORES = 8
N_FULL = 8192          # rows of x (and of G)
M_FULL = 8192          # rows of y (cols of G)
D = 512                # feature dim (contraction)
MC = N_FULL // N_CORES # 1024 rows of x per core
P = 128                # SBUF partitions
KT = D // P            # 4 k-subtiles of 128
MT = MC // P           # 8 m-tiles per core
CW = 2048              # chunk width (ACT/DVE/psum-slot granularity)
JC = M_FULL // CW      # 4 j-chunks
NN = CW // 512         # 4 matmul slices of 512 per chunk
G = JC * MT            # 32 chunks per core
XS = 16.0              # fp8 input scale (folded out via ACT scale)

_cache = {}


def _build_program(scale_imm: float, out_fp8: bool, dve_mode: str):
    """Raw-Bass build: explicit per-engine programs + hand-rolled semaphores."""
    from contextlib import ExitStack, contextmanager

    import concourse.bass as bass
    import concourse.mybir as mybir
    from concourse import bacc

    class _NoBarrierBlock(bass.BassBlock):
        """BassBlock whose exit emits per-engine drains but no all-engine
        barrier; cross-engine ordering is fully covered by our semaphores."""

        def __exit__(self, exc_type, exc_val, exc_tb):
            if exc_type is not None:
                return
            for engine, last_body in self.last_body.items():
                with self.bass.body(last_body, parent=self.bass.cur_bb,
                                    allow_existing_parent=True):
                    engine.br(self.end_bb)
            self.bass.switch_bb(self.end_bb)
            gpsimd_type = self.bass.gpsimd.engine
            for eng_type, eng in self.bass.engines.items():
                if eng_type == gpsimd_type:
                    continue
                dr = mybir.InstDrain(
                    name=self.bass.get_next_instruction_name(),
                    ins=[], outs=[], bass_is_fusable=False)
                dr.engine = eng_type
                eng.add_instruction(dr)

    @contextmanager
    def _no_barrier_block(nc):
        assert nc.cur_block is None
        blk = _NoBarrierBlock(nc, f"block_{nc.next_id()}")
        nc.cur_block = blk
        try:
            with blk:
                yield blk
        finally:
            nc.cur_block = None

    NWARM = 40
    O_SLOTS = 4            # ACT output staging slots (bf16)
    O2_SLOTS = 6           # DVE output staging slots (fp8/bf16)
    fp8 = mybir.dt.float8e4
    odt = mybir.dt.float8e4 if out_fp8 else mybir.dt.bfloat16
    two_step = dve_mode == "mul16copy8" and out_fp8

    nc = bacc.Bacc("TRN2", target_bir_lowering=False, debug=False,
                   num_devices=N_CORES)

    x_d = nc.dram_tensor("x8", [P, MT * KT, P], fp8,
                         kind="ExternalInput").ap()
    y_d = nc.dram_tensor("y8", [P, JC * KT, CW], fp8,
                         kind="ExternalInput").ap()
    c_d = nc.dram_tensor("cb", [P, M_FULL], mybir.dt.bfloat16,
                         kind="ExternalInput").ap()
    x2_d = nc.dram_tensor("x2b", [P, MT], mybir.dt.float32,
                          kind="ExternalInput").ap()
    out_d = nc.dram_tensor("out", [MC, M_FULL], odt,
                           kind="ExternalOutput").ap()

    with ExitStack() as ctx:
        ec = ctx.enter_context
        x_sb = ec(nc.sbuf_tensor([P, MT * KT, P], fp8))
        y_sb = ec(nc.sbuf_tensor([P, JC * KT, CW], fp8))
        c_sb = ec(nc.sbuf_tensor([P, M_FULL], mybir.dt.bfloat16))
        x2_sb = ec(nc.sbuf_tensor([P, MT], mybir.dt.float32))
        scr_sb = ec(nc.sbuf_tensor([P, 2 * P], mybir.dt.bfloat16))
        o_sb = ec(nc.sbuf_tensor([P, O_SLOTS, CW], mybir.dt.bfloat16))
        o2_sb = ec(nc.sbuf_tensor([P, O2_SLOTS, CW], odt))
        o3_sb = (ec(nc.sbuf_tensor("o3_sb", [P, CW], mybir.dt.bfloat16))
                 if two_step else None)
        ps = ec(nc.psum_tensor([P, 2, CW], mybir.dt.float32))

        s_scr = ec(nc.semaphore(name="s_scr"))
        s_xa = ec(nc.semaphore(name="s_xa"))
        s_xb = ec(nc.semaphore(name="s_xb"))
        s_x2 = ec(nc.semaphore(name="s_x2"))
        s_y = [ec(nc.semaphore(name=f"s_y{i}")) for i in range(JC)]
        s_y0k = [ec(nc.semaphore(name=f"s_y0k{i}")) for i in range(KT)]
        s_c = [ec(nc.semaphore(name=f"s_c{i}")) for i in range(JC)]
        s_mm = ec(nc.semaphore(name="s_mm"))
        s_act = ec(nc.semaphore(name="s_act"))
        s_dve = ec(nc.semaphore(name="s_dve"))
        s_osl = [ec(nc.semaphore(name=f"s_osl{i}")) for i in range(O2_SLOTS)]

        def lhsT(m, ko):
            return x_sb[:, m * KT + 2 * ko:m * KT + 2 * ko + 2, :]

        def rhs(jc, ko, nn):
            return y_sb[:, jc * KT + 2 * ko:jc * KT + 2 * ko + 2,
                        nn * 512:(nn + 1) * 512]

        # plan: chunks 0 and G-1 are split into NN 512-wide pieces so the
        # pipeline head fills (and the tail drains) at piece granularity.
        # cum[(g, i)] = cumulative semaphore count after piece i of chunk g
        # (the same piece structure is used for s_mm, s_act and s_dve).
        pieces = [NN if g in (0, G - 1) else 1 for g in range(G)]
        cum, tot = {}, {}
        _c = 0
        for g in range(G):
            for i in range(pieces[g]):
                _c += 1
                cum[(g, i)] = _c
            tot[g] = _c
        # s_osl[sl] count accumulated by out-DMAs of chunks < g on slot sl
        osl_before = {}
        _oc = [0] * O2_SLOTS
        for g in range(G):
            osl_before[g] = _oc[g % O2_SLOTS]
            _oc[g % O2_SLOTS] += 16 * pieces[g]

        def psl(g, i):
            w = CW // pieces[g]
            return slice(i * w, (i + 1) * w)

        with _no_barrier_block(nc) as block:

            @block.sync
            def _(sync):
                # startup set, in critical-path order; y chunk 0 is split
                # per k-subtile so its pieces ride parallel DMA queues, and
                # x's m0 block is pulled ahead so chunk 0 isn't gated on
                # the full x image.
                for kt in range(KT):
                    sync.dma_start(out=y_sb[:, kt:kt + 1, :],
                                   in_=y_d[:, kt:kt + 1, :]
                                   ).then_inc(s_y0k[kt], 16)
                sync.dma_start(out=x_sb[:, 0:KT, :],
                               in_=x_d[:, 0:KT, :]).then_inc(s_xa, 16)
                sync.dma_start(out=x_sb[:, KT:, :],
                               in_=x_d[:, KT:, :]).then_inc(s_xb, 16)
                sync.dma_start(out=x2_sb[:], in_=x2_d).then_inc(s_x2, 16)
                sync.dma_start(out=c_sb[:, 0:CW],
                               in_=c_d[:, 0:CW]).then_inc(s_c[0], 16)
                sync.dma_start(out=y_sb[:, KT:2 * KT, :],
                               in_=y_d[:, KT:2 * KT, :]).then_inc(s_y[1], 16)
                sync.dma_start(out=c_sb[:, CW:2 * CW],
                               in_=c_d[:, CW:2 * CW]).then_inc(s_c[1], 16)
                for g in range(G):
                    jc, m = g // MT, g % MT
                    # just-in-time prefetch of later y/c chunks
                    if g == 2:
                        sync.dma_start(out=y_sb[:, 2 * KT:3 * KT, :],
                                       in_=y_d[:, 2 * KT:3 * KT, :]
                                       ).then_inc(s_y[2], 16)
                    if g == 4:
                        sync.dma_start(out=c_sb[:, 2 * CW:3 * CW],
                                       in_=c_d[:, 2 * CW:3 * CW]
                                       ).then_inc(s_c[2], 16)
                    if g == 10:
                        sync.dma_start(out=y_sb[:, 3 * KT:4 * KT, :],
                                       in_=y_d[:, 3 * KT:4 * KT, :]
                                       ).then_inc(s_y[3], 16)
                    if g == 12:
                        sync.dma_start(out=c_sb[:, 3 * CW:4 * CW],
                                       in_=c_d[:, 3 * CW:4 * CW]
                                       ).then_inc(s_c[3], 16)
                    sl = g % O2_SLOTS
                    msl = slice(m * P, (m + 1) * P)
                    for i in range(pieces[g]):
                        sync.wait_ge(s_dve, cum[(g, i)])
                        w = psl(g, i)
                        sync.dma_start(
                            out=out_d[msl, jc * CW + w.start:
                                      jc * CW + w.stop],
                            in_=o2_sb[:, sl, w]).then_inc(s_osl[sl], 16)
                # the end-of-block DRAIN quiesces the DGE queues

            @block.tensor
            def _(tensor):
                # PE warm-up: keep the HAM activity window busy while the
                # startup DMAs land, so real matmuls run at 2.4 GHz.
                tensor.wait_ge(s_scr, 1)
                for _ in range(NWARM):
                    tensor.matmul(ps[:, 0, 0:P], lhsT=scr_sb[:, P:2 * P],
                                  rhs=scr_sb[:, 0:P], start=True, stop=True)
                tensor.wait_ge(s_xa, 16)
                for g in range(G):
                    jc, m = g // MT, g % MT
                    sl = g % 2
                    if g == 1:
                        tensor.wait_ge(s_xb, 16)   # rest of the x image
                    if g >= 2:
                        tensor.wait_ge(s_act, tot[g - 2])   # psum slot free
                    if m == 0 and jc > 0:
                        tensor.wait_ge(s_y[jc], 16)
                    if g == 0:
                        # nn-outer so the first ACT piece unblocks after 2
                        # MMs; per-kt gating because the 4 startup y DMAs
                        # ride parallel queues and can land out of order
                        for nn in range(NN):
                            for ko in range(2):
                                if nn == 0:
                                    tensor.wait_ge(s_y0k[2 * ko], 16)
                                    tensor.wait_ge(s_y0k[2 * ko + 1], 16)
                                inst = tensor.matmul(
                                    ps[:, sl, nn * 512:(nn + 1) * 512],
                                    lhsT=lhsT(m, ko),
                                    rhs=rhs(jc, ko, nn),
                                    start=(ko == 0),
                                    stop=(ko == 1),
                                    perf_mode=mybir.MatmulPerfMode.DoubleRow,
                                )
                            inst.then_inc(s_mm, 1)
                    else:
                        for ko in range(2):
                            for nn in range(NN):
                                inst = tensor.matmul(
                                    ps[:, sl, nn * 512:(nn + 1) * 512],
                                    lhsT=lhsT(m, ko),
                                    rhs=rhs(jc, ko, nn),
                                    start=(ko == 0),
                                    stop=(ko == 1),
                                    perf_mode=mybir.MatmulPerfMode.DoubleRow,
                                )
                        if pieces[g] == 1:
                            inst.then_inc(s_mm, 1)
                        else:
                            inst.then_inc(s_mm, pieces[g])

            @block.scalar
            def _(scalar):
                # dummy activation so the one-time exp table load (~2.7us)
                # overlaps the startup DMAs instead of the first real chunk.
                # o_sb slot 0 is private to this engine until chunk 0.
                scalar.activation(o_sb[:, 0, 0:2], o_sb[:, 0, 0:2],
                                  mybir.ActivationFunctionType.Exp)
                scalar.wait_ge(s_x2, 16)
                for g in range(G):
                    jc, m = g // MT, g % MT
                    osl = g % O_SLOTS
                    if g >= O_SLOTS:
                        scalar.wait_ge(s_dve, tot[g - O_SLOTS])  # o slot free
                    for i in range(pieces[g]):
                        scalar.wait_ge(s_mm, cum[(g, i)])
                        w = psl(g, i)
                        scalar.activation(
                            o_sb[:, osl, w], ps[:, g % 2, w],
                            mybir.ActivationFunctionType.Exp,
                            bias=x2_sb[:, m:m + 1],
                            scale=float(scale_imm)).then_inc(s_act, 1)

            @block.vector
            def _(vector):
                vector.memset(scr_sb[:], 0.0).then_inc(s_scr, 1)
                for g in range(G):
                    jc, m = g // MT, g % MT
                    osl = g % O_SLOTS
                    sl = g % O2_SLOTS
                    if g >= O2_SLOTS:
                        vector.wait_ge(s_osl[sl], osl_before[g])  # slot free
                    if m == 0:
                        vector.wait_ge(s_c[jc], 16)
                    csl = c_sb[:, jc * CW:(jc + 1) * CW]
                    for i in range(pieces[g]):
                        vector.wait_ge(s_act, cum[(g, i)])
                        w = psl(g, i)
                        if two_step:
                            # keep the multiply in the bf16 2x mode; the
                            # fp8 downcast rides the copy uop
                            vector.tensor_mul(o3_sb[:, w], o_sb[:, osl, w],
                                              csl[:, w])
                            vector.tensor_copy(
                                o2_sb[:, sl, w],
                                o3_sb[:, w]).then_inc(s_dve, 1)
                        else:
                            vector.tensor_mul(
                                o2_sb[:, sl, w], o_sb[:, osl, w],
                                csl[:, w]).then_inc(s_dve, 1)

        nc.compile()
    return nc


def _pack_xT(x_8: np.ndarray) -> np.ndarray:
    """[MC, D] fp8 -> SBUF image [128, MT*KT, 128]; k-subtile kt of m-tile m
    at dim1 index m*KT+kt with element [p, ., c] = x[m*128 + c, kt*128 + p]."""
    mcc, d = x_8.shape
    mt, kt = mcc // P, d // P
    a = x_8.reshape(mt, P, kt, P)          # [m, c, kt, p]
    a = a.transpose(3, 0, 2, 1)            # [p, m, kt, c]
    return np.ascontiguousarray(a.reshape(P, mt * kt, P))


def _pack_yT(y_8: np.ndarray) -> np.ndarray:
    """[M, D] fp8 -> SBUF image [128, JC*KT, CW]; k-subtile kt of j-chunk jc
    at dim1 index jc*KT+kt with element [p, ., c] = y[jc*CW + c, kt*128 + p]."""
    m, d = y_8.shape
    jc, kt = m // CW, d // P
    a = y_8.reshape(jc, CW, kt, P)         # [jc, c, kt, p]
    a = a.transpose(3, 0, 2, 1)            # [p, jc, kt, c]
    return np.ascontiguousarray(a.reshape(P, jc * kt, CW))


def kernel(x: np.ndarray, y: np.ndarray, gamma: np.ndarray) -> np.ndarray:
    from concourse.bass_utils import run_bass_kernel_spmd

    x = np.asarray(x, dtype=np.float32)
    y = np.asarray(y, dtype=np.float32)
    g = float(np.asarray(gamma))

    n, d = x.shape
    m = y.shape[0]
    assert (n, d, m) == (N_FULL, D, M_FULL), (n, d, m)

    out_fp8 = os.environ.get("RBF_OUT", "fp8") == "fp8"
    dve_mode = os.environ.get("RBF_DVE", "mul16copy8")
    scale_imm = 2.0 * g / (XS * XS)
    key = (g, out_fp8, dve_mode)
    if key not in _cache:
        _cache.clear()
        _cache[key] = _build_program(scale_imm, out_fp8, dve_mode)
    nc = _cache[key]

    # host-side prep (O(N*D), ~0.01% of kernel FLOPs)
    f8 = ml_dtypes.float8_e4m3fn
    x8 = np.clip(x * XS, -240.0, 240.0).astype(f8)
    y8 = np.clip(y * XS, -240.0, 240.0).astype(f8)
    y_img = _pack_yT(y8)
    x2 = np.einsum("nd,nd->n", x, x, dtype=np.float64)
    y2 = np.einsum("md,md->m", y, y, dtype=np.float64)
    c_row = np.exp(-g * y2).astype(ml_dtypes.bfloat16)
    c_rep = np.ascontiguousarray(np.broadcast_to(c_row[None, :], (P, m)))

    in_maps = []
    for c in range(N_CORES):
        sl = slice(c * MC, (c + 1) * MC)
        x2_c = np.ascontiguousarray(
            (-g * x2[sl]).astype(np.float32).reshape(MT, P).T)   # [128, MT]
        in_maps.append({"x8": _pack_xT(x8[sl]), "y8": y_img,
                        "cb": c_rep, "x2b": x2_c})

    trace = bool(int(os.environ.get("RBF_TRACE", "0")))
    res = run_bass_kernel_spmd(nc, in_maps, core_ids=list(range(N_CORES)),
                               trace=trace)
    global LAST_RESULTS
    LAST_RESULTS = res
    return np.concatenate(
        [r["out"].astype(np.float32) for r in res.results], axis=0)


LAST_RESULTS = None
